# revision 55
# baseline (speedup 1.0000x reference)
"""AvgLineExtractor (segment mean over line buckets) on 8 Trainium2 cores.

Full inputs -> full outputs. Data-parallel over batch: 16 batches / 8 cores
= 2 batches per core; segments never cross the batch dim so there is no
cross-core communication.

Per-core algorithm (per batch):
  - tokens live on SBUF partitions in blocked order: token = p*32 + j
    (p = partition, j = 0..31), so feature DMAs are big contiguous chunks.
  - DVE ops build the one-hot tiles, one half-batch at a time:
    E[p, j*128 + m] = (idx[p*32+j] == m+1)  (fp32r; 0/1 exact).
  - sums:   PSUM[bucket, dim] += E_j^T @ feat_tile_j, accumulated over the
    32 token tiles on the PE. Features are DMA'd as fp32 then rounded to
    fp32r (fp32r streams the PE at 1 cycle/row vs fp32's 4); the rounding
    alternates between the scalar (ACT) and vector engines so neither
    serializes the pipeline.
    Bucket 0 (padding) matches no one-hot column and drops out naturally.
  - counts: row vector ones^T @ E in N=512 stationary-weight matmuls (no
    per-tile LDWEIGHTS), folded 4->1 on the DVE and transposed to
    [bucket, 1] with a 1x1-identity PE transpose.
  - finalize: out = sums * 1/(1+counts); mask = (counts != 0) packed as
    fp32 column 512 of the output tile so one DMA writes everything.

Engine/semaphore layout notes: walrus allows one sync-wait per
instruction; Bacc legalizes multi-waits into EVENT_SEMAPHORE chains which
cost ~0.8-1.7us each on the SP sequencer (cheap on engine sequencers).
The kernel-tail drain waits every live semaphore through such a chain, so
the design minimizes semaphore count: all DMAs ride HWDGE queues (6
total), no SWDGE semaphores.
"""

import numpy as np
from contextlib import ExitStack

import bass_rust
import concourse.bacc as bacc
import concourse.bass as bass
import concourse.mybir as mybir
import concourse.tile as tile
from concourse.bass_utils import run_bass_kernel_spmd

N_CORES = 8
BSZ, SEQ, DIM = 16, 4096, 512
B_PER_CORE = BSZ // N_CORES  # 2
ML = 128                     # MAX_LINES (buckets 1..128 kept, bucket 0 dropped)
P = 128                      # SBUF partitions
TILES = SEQ // P             # 32 token tiles per batch
CHUNK = 4                    # token tiles per feature DMA (512 KiB)
EHALF = TILES // 4           # one-hot tiles built per DVE op
ODIM = DIM + 1               # output row: 512 features + mask flag

_nc_cache = None


def _build_nc() -> bass.Bass:
    # Bacc (not raw Bass): its finalize() runs generate_event_semaphores,
    # which legalizes multi-semaphore waits for the TRN2 ISA.
    nc = bacc.Bacc(None, target_bir_lowering=False)
    feat = nc.dram_tensor(
        "feat", [B_PER_CORE, SEQ, DIM], mybir.dt.float32, kind="ExternalInput"
    )
    idx = nc.dram_tensor(
        "idx", [B_PER_CORE, SEQ], mybir.dt.int32, kind="ExternalInput"
    )
    # laid out [bucket, batch, dim] so the single output DMA is contiguous
    # per partition; the host transposes back
    out_feat = nc.dram_tensor(
        "out_feat", [ML, B_PER_CORE, ODIM], mybir.dt.float32, kind="ExternalOutput"
    )

    with ExitStack() as ctx:
        tc = ctx.enter_context(tile.TileContext(nc))
        const = ctx.enter_context(tc.tile_pool(name="const", bufs=1))
        fpool = ctx.enter_context(tc.tile_pool(name="feat", bufs=8))
        frpool = ctx.enter_context(tc.tile_pool(name="featr", bufs=6))
        epool = ctx.enter_context(tc.tile_pool(name="onehot", bufs=4))
        spool = ctx.enter_context(tc.tile_pool(name="small", bufs=2))
        opool = ctx.enter_context(tc.tile_pool(name="outs", bufs=1))
        psum = ctx.enter_context(tc.tile_pool(name="psum", bufs=2, space="PSUM"))

        # idx DMA first (HWDGE; everything at the start waits on it)
        # idx_i[p, b*TILES + j] = idx[b, p*32 + j]
        ir = idx.rearrange("b (p j) -> p b j", p=P)
        idx_i = spool.tile([P, B_PER_CORE * TILES], mybir.dt.int32)
        nc.sync.dma_start(
            out=idx_i[:].rearrange("p (b j) -> p b j", b=B_PER_CORE), in_=ir
        )
        # copy on gpsimd (Pool) so E's deps (idx_f + iota_f) sit on one
        # semaphore
        idx_f = spool.tile([P, B_PER_CORE * TILES], mybir.dt.float32)
        nc.gpsimd.tensor_copy(idx_f[:], idx_i[:])

        # iota_f[p, m] = m + 1 for all p (values <= 128, exact in fp32)
        iota_f = const.tile([P, ML], mybir.dt.float32)
        nc.gpsimd.iota(
            iota_f[:],
            pattern=[[1, ML]],
            base=1,
            channel_multiplier=0,
            allow_small_or_imprecise_dtypes=True,
        )
        ones = const.tile([P, 1], mybir.dt.float32)
        nc.gpsimd.memset(ones[:], 1.0)
        # fp32r copy doubles as the required "rounded" producer
        ones_r = const.tile([P, 1], mybir.dt.float32r)
        nc.vector.tensor_copy(ones_r[:], ones[:])

        # outputs for both batches accumulate here; one DMA at the end
        of_all = opool.tile([P, B_PER_CORE * ODIM], mybir.dt.float32)

        batch_state = []  # (cnt_row, psum_sums, last_mm) per batch
        for b in range(B_PER_CORE):
            # fr[p, j*DIM + d] = feat[b, p*32 + j, d]  (contiguous / partition)
            fr = feat[b].rearrange("(p j) d -> p (j d)", p=P)

            # ---- one-hot + counts, half a batch at a time ----
            psum_cnt_row = psum.tile([1, 4 * ML], mybir.dt.float32)
            halves = []
            for h in range(TILES // EHALF):
                E = epool.tile([P, EHALF * ML], mybir.dt.float32r)
                j0 = h * EHALF
                nc.vector.tensor_tensor(
                    out=E[:].rearrange("p (j m) -> p j m", m=ML),
                    in0=idx_f[
                        :, b * TILES + j0 : b * TILES + j0 + EHALF, None
                    ].to_broadcast([P, EHALF, ML]),
                    in1=iota_f[:, None, :].to_broadcast([P, EHALF, ML]),
                    op=mybir.AluOpType.is_equal,
                )
                halves.append(E)
                for g in range(EHALF * ML // (4 * ML)):
                    gi = h * (EHALF // 4) + g
                    nc.tensor.matmul(
                        out=psum_cnt_row[:],
                        lhsT=ones_r[:, :1],
                        rhs=E[:, g * 4 * ML : (g + 1) * 4 * ML],
                        start=(gi == 0),
                        stop=(gi == TILES // 4 - 1),
                    )
            # fold the 4 residue classes: cnt_row[0, m] = count of bucket m+1
            cnt_row = spool.tile([1, ML], mybir.dt.float32)
            nc.vector.tensor_reduce(
                out=cnt_row[:],
                in_=psum_cnt_row[:].rearrange("p (q m) -> p m q", m=ML),
                axis=mybir.AxisListType.X,
                op=mybir.AluOpType.add,
            )
            # ---- sums: E_j^T @ feat_j accumulated over 32 token tiles ----
            # fine-grained chunks: the cast and the matmuls trail the DMA
            # stream by ~1us instead of a whole multi-MiB chunk, so the
            # post-last-byte critical path is short. Tile wraps DMA-queue
            # semaphore slots, so more DMAs don't grow the drain's wait list.
            chunks = [CHUNK] * (TILES // CHUNK)
            if b == B_PER_CORE - 1:
                chunks = [CHUNK] * (TILES // CHUNK - 1) + [2, 2]
            psum_sums = psum.tile([P, DIM], mybir.dt.float32)
            j = 0
            last_mm = None
            for csize in chunks:
                ft = fpool.tile([P, CHUNK * DIM], mybir.dt.float32)
                nc.sync.dma_start(
                    out=ft[:, : csize * DIM],
                    in_=fr[:, j * DIM : (j + csize) * DIM],
                )
                # round to fp32r in a distinct tile (verifier requirement);
                # DVE only — ACT crashes producing fp32r, GpSimd is ~4x slower
                ftr = frpool.tile([P, CHUNK * DIM], mybir.dt.float32r)
                nc.vector.tensor_copy(ftr[:, : csize * DIM], ft[:, : csize * DIM])
                for jj in range(csize):
                    E = halves[j // EHALF]
                    jh = j % EHALF
                    last_mm = nc.tensor.matmul(
                        out=psum_sums[:],
                        lhsT=E[:, jh * ML : (jh + 1) * ML],
                        rhs=ftr[:, jj * DIM : (jj + 1) * DIM],
                        start=(j == 0),
                        stop=(j == TILES - 1),
                    )
                    j += 1
            batch_state.append((cnt_row, psum_sums, last_mm))

        # ---- count reshape + finalize ----
        # cnt_row -> [ML, 1] via a tiny HWDGE DMA reshape: no PE transpose
        # (a transpose-mode switch stalls following matmuls ~10us), runs
        # mid-kernel well off the critical path, and queue slots wrap so it
        # adds no drain semaphores.
        for b in range(B_PER_CORE):
            cnt_row, psum_sums, _ = batch_state[b]
            psum_cnt = spool.tile([P, 1], mybir.dt.float32)
            nc.sync.dma_start(out=psum_cnt[:], in_=cnt_row[:])

            denom = spool.tile([P, 1], mybir.dt.float32)
            nc.vector.tensor_scalar_add(denom[:], psum_cnt[:], 1.0)
            recip = spool.tile([P, 1], mybir.dt.float32)
            nc.vector.reciprocal(recip[:], denom[:])
            # mask as fp32 0/1 in the extra output column
            nc.vector.tensor_scalar(
                out=of_all[:, b * ODIM + DIM : b * ODIM + DIM + 1],
                in0=psum_cnt[:],
                scalar1=0.0,
                scalar2=None,
                op0=mybir.AluOpType.not_equal,
            )
            # multiply straight out of PSUM (a 2-semaphore wait costs one
            # cheap engine-sequencer event-semaphore, less than a copy)
            nc.vector.tensor_scalar_mul(
                of_all[:, b * ODIM : b * ODIM + DIM], psum_sums[:], recip[:]
            )

        # single contiguous output DMA: out_feat[m, b, :] <- of_all[m, b*ODIM:]
        nc.sync.dma_start(
            out=out_feat.rearrange("m b d -> m (b d)"), in_=of_all[:]
        )


    nc.finalize()
    return nc


def get_nc() -> bass.Bass:
    global _nc_cache
    if _nc_cache is None:
        _nc_cache = _build_nc()
    return _nc_cache


def make_in_maps(token_features: np.ndarray, line_idxes: np.ndarray):
    idx = np.ascontiguousarray(line_idxes.reshape(BSZ, SEQ), dtype=np.int32)
    feat = np.ascontiguousarray(token_features, dtype=np.float32)
    in_maps = []
    for c in range(N_CORES):
        sl = slice(c * B_PER_CORE, (c + 1) * B_PER_CORE)
        in_maps.append(
            {
                "feat": np.ascontiguousarray(feat[sl]),
                "idx": np.ascontiguousarray(idx[sl]),
            }
        )
    return in_maps


def assemble(results):
    # per-core result is [ML, B_PER_CORE, ODIM]; reorder to [bsz, ML, ODIM]
    out = np.concatenate(
        [r["out_feat"].transpose(1, 0, 2) for r in results], axis=0
    )
    line_features = np.ascontiguousarray(out[:, :, :DIM], dtype=np.float32)
    line_mask = out[:, :, DIM] != 0.0
    return line_features, line_mask


def kernel(**inputs) -> tuple[np.ndarray, np.ndarray]:
    token_features = np.asarray(inputs["token_features"])
    line_idxes = np.asarray(inputs["line_idxes"])
    nc = get_nc()
    in_maps = make_in_maps(token_features, line_idxes)
    res = run_bass_kernel_spmd(nc, in_maps, core_ids=list(range(N_CORES))).results
    return assemble(res)


# revision 56
# speedup vs baseline: 1.1603x; 1.1603x over previous
"""AvgLineExtractor (segment mean over line buckets) on 8 Trainium2 cores.

Full inputs -> full outputs. Data-parallel over batch: 16 batches / 8 cores
= 2 batches per core; segments never cross the batch dim so there is no
cross-core communication.

Per-core algorithm (per batch):
  - tokens live on SBUF partitions in blocked order: token = p*32 + j
    (p = partition, j = 0..31), so feature DMAs are big contiguous chunks.
  - DVE ops build the one-hot tiles, one half-batch at a time:
    E[p, j*128 + m] = (idx[p*32+j] == m+1)  (fp32r; 0/1 exact).
  - sums:   PSUM[bucket, dim] += E_j^T @ feat_tile_j, accumulated over the
    32 token tiles on the PE. Features are DMA'd as fp32 then rounded to
    fp32r (fp32r streams the PE at 1 cycle/row vs fp32's 4); the rounding
    alternates between the scalar (ACT) and vector engines so neither
    serializes the pipeline.
    Bucket 0 (padding) matches no one-hot column and drops out naturally.
  - counts: row vector ones^T @ E in N=512 stationary-weight matmuls (no
    per-tile LDWEIGHTS), folded 4->1 on the DVE and transposed to
    [bucket, 1] with a 1x1-identity PE transpose.
  - finalize: out = sums * 1/(1+counts); mask = (counts != 0) packed as
    fp32 column 512 of the output tile so one DMA writes everything.

Engine/semaphore layout notes: walrus allows one sync-wait per
instruction; Bacc legalizes multi-waits into EVENT_SEMAPHORE chains which
cost ~0.8-1.7us each on the SP sequencer (cheap on engine sequencers).
The kernel-tail drain waits every live semaphore through such a chain, so
the design minimizes semaphore count: all DMAs ride HWDGE queues (6
total), no SWDGE semaphores.
"""

import numpy as np
from contextlib import ExitStack

import bass_rust
import concourse.bacc as bacc
import concourse.bass as bass
import concourse.mybir as mybir
import concourse.tile as tile
from concourse.bass_utils import run_bass_kernel_spmd

N_CORES = 8
BSZ, SEQ, DIM = 16, 4096, 512
B_PER_CORE = BSZ // N_CORES  # 2
ML = 128                     # MAX_LINES (buckets 1..128 kept, bucket 0 dropped)
P = 128                      # SBUF partitions
TILES = SEQ // P             # 32 token tiles per batch
CHUNK = 4                    # token tiles per feature DMA (512 KiB)
EHALF = TILES // 2           # one-hot tiles built per DVE op
ODIM = DIM + 1               # output row: 512 features + mask flag

_nc_cache = None


def _build_nc() -> bass.Bass:
    # Bacc (not raw Bass): its finalize() runs generate_event_semaphores,
    # which legalizes multi-semaphore waits for the TRN2 ISA.
    nc = bacc.Bacc(None, target_bir_lowering=False)
    feat = nc.dram_tensor(
        "feat", [B_PER_CORE, SEQ, DIM], mybir.dt.float32, kind="ExternalInput"
    )
    idx = nc.dram_tensor(
        "idx", [B_PER_CORE, SEQ], mybir.dt.int32, kind="ExternalInput"
    )
    # laid out [bucket, batch, dim] so the single output DMA is contiguous
    # per partition; the host transposes back
    out_feat = nc.dram_tensor(
        "out_feat", [ML, B_PER_CORE, ODIM], mybir.dt.float32, kind="ExternalOutput"
    )

    with ExitStack() as ctx:
        tc = ctx.enter_context(tile.TileContext(nc))
        const = ctx.enter_context(tc.tile_pool(name="const", bufs=1))
        fpool = ctx.enter_context(tc.tile_pool(name="feat", bufs=8))
        frpool = ctx.enter_context(tc.tile_pool(name="featr", bufs=6))
        epool = ctx.enter_context(tc.tile_pool(name="onehot", bufs=2))
        spool = ctx.enter_context(tc.tile_pool(name="small", bufs=2))
        opool = ctx.enter_context(tc.tile_pool(name="outs", bufs=1))
        psum = ctx.enter_context(tc.tile_pool(name="psum", bufs=2, space="PSUM"))

        # idx DMA first (HWDGE; everything at the start waits on it)
        # idx_i[p, b*TILES + j] = idx[b, p*32 + j]
        ir = idx.rearrange("b (p j) -> p b j", p=P)
        idx_i = spool.tile([P, B_PER_CORE * TILES], mybir.dt.int32)
        nc.sync.dma_start(
            out=idx_i[:].rearrange("p (b j) -> p b j", b=B_PER_CORE), in_=ir
        )
        # copy on gpsimd (Pool) so E's deps (idx_f + iota_f) sit on one
        # semaphore
        idx_f = spool.tile([P, B_PER_CORE * TILES], mybir.dt.float32)
        nc.gpsimd.tensor_copy(idx_f[:], idx_i[:])

        # iota_f[p, m] = m + 1 for all p (values <= 128, exact in fp32)
        iota_f = const.tile([P, ML], mybir.dt.float32)
        nc.gpsimd.iota(
            iota_f[:],
            pattern=[[1, ML]],
            base=1,
            channel_multiplier=0,
            allow_small_or_imprecise_dtypes=True,
        )
        ones = const.tile([P, 1], mybir.dt.float32)
        nc.gpsimd.memset(ones[:], 1.0)
        # fp32r copy doubles as the required "rounded" producer
        ones_r = const.tile([P, 1], mybir.dt.float32r)
        nc.vector.tensor_copy(ones_r[:], ones[:])

        # outputs for both batches accumulate here; one DMA at the end
        of_all = opool.tile([P, B_PER_CORE * ODIM], mybir.dt.float32)

        batch_state = []  # (cnt_row, psum_sums, last_mm) per batch
        for b in range(B_PER_CORE):
            # fr[p, j*DIM + d] = feat[b, p*32 + j, d]  (contiguous / partition)
            fr = feat[b].rearrange("(p j) d -> p (j d)", p=P)

            # ---- one-hot + counts, half a batch at a time ----
            psum_cnt_row = psum.tile([1, 4 * ML], mybir.dt.float32)
            halves = []
            for h in range(TILES // EHALF):
                E = epool.tile([P, EHALF * ML], mybir.dt.float32r)
                j0 = h * EHALF
                nc.vector.tensor_tensor(
                    out=E[:].rearrange("p (j m) -> p j m", m=ML),
                    in0=idx_f[
                        :, b * TILES + j0 : b * TILES + j0 + EHALF, None
                    ].to_broadcast([P, EHALF, ML]),
                    in1=iota_f[:, None, :].to_broadcast([P, EHALF, ML]),
                    op=mybir.AluOpType.is_equal,
                )
                halves.append(E)
                for g in range(EHALF * ML // (4 * ML)):
                    gi = h * (EHALF // 4) + g
                    nc.tensor.matmul(
                        out=psum_cnt_row[:],
                        lhsT=ones_r[:, :1],
                        rhs=E[:, g * 4 * ML : (g + 1) * 4 * ML],
                        start=(gi == 0),
                        stop=(gi == TILES // 4 - 1),
                    )
            # fold the 4 residue classes: cnt_row[0, m] = count of bucket m+1
            cnt_row = spool.tile([1, ML], mybir.dt.float32)
            nc.vector.tensor_reduce(
                out=cnt_row[:],
                in_=psum_cnt_row[:].rearrange("p (q m) -> p m q", m=ML),
                axis=mybir.AxisListType.X,
                op=mybir.AluOpType.add,
            )
            # ---- sums: E_j^T @ feat_j accumulated over 32 token tiles ----
            # fine-grained chunks: the cast and the matmuls trail the DMA
            # stream by ~1us instead of a whole multi-MiB chunk, so the
            # post-last-byte critical path is short. Tile wraps DMA-queue
            # semaphore slots, so more DMAs don't grow the drain's wait list.
            chunks = [CHUNK] * (TILES // CHUNK)
            if b == B_PER_CORE - 1:
                chunks = [CHUNK] * (TILES // CHUNK - 1) + [2, 2]
            psum_sums = psum.tile([P, DIM], mybir.dt.float32)
            j = 0
            last_mm = None
            for csize in chunks:
                ft = fpool.tile([P, CHUNK * DIM], mybir.dt.float32)
                nc.sync.dma_start(
                    out=ft[:, : csize * DIM],
                    in_=fr[:, j * DIM : (j + csize) * DIM],
                )
                # round to fp32r in a distinct tile (verifier requirement);
                # DVE only — ACT crashes producing fp32r, GpSimd is ~4x slower
                ftr = frpool.tile([P, CHUNK * DIM], mybir.dt.float32r)
                nc.vector.tensor_copy(ftr[:, : csize * DIM], ft[:, : csize * DIM])
                for jj in range(csize):
                    E = halves[j // EHALF]
                    jh = j % EHALF
                    last_mm = nc.tensor.matmul(
                        out=psum_sums[:],
                        lhsT=E[:, jh * ML : (jh + 1) * ML],
                        rhs=ftr[:, jj * DIM : (jj + 1) * DIM],
                        start=(j == 0),
                        stop=(j == TILES - 1),
                    )
                    j += 1
            batch_state.append((cnt_row, psum_sums, last_mm))

        # ---- count reshape + finalize ----
        # cnt_row -> [ML, 1] via a tiny HWDGE DMA reshape: no PE transpose
        # (a transpose-mode switch stalls following matmuls ~10us), runs
        # mid-kernel well off the critical path, and queue slots wrap so it
        # adds no drain semaphores.
        for b in range(B_PER_CORE):
            cnt_row, psum_sums, _ = batch_state[b]
            psum_cnt = spool.tile([P, 1], mybir.dt.float32)
            nc.sync.dma_start(out=psum_cnt[:], in_=cnt_row[:])

            denom = spool.tile([P, 1], mybir.dt.float32)
            nc.vector.tensor_scalar_add(denom[:], psum_cnt[:], 1.0)
            recip = spool.tile([P, 1], mybir.dt.float32)
            nc.vector.reciprocal(recip[:], denom[:])
            # mask as fp32 0/1 in the extra output column
            nc.vector.tensor_scalar(
                out=of_all[:, b * ODIM + DIM : b * ODIM + DIM + 1],
                in0=psum_cnt[:],
                scalar1=0.0,
                scalar2=None,
                op0=mybir.AluOpType.not_equal,
            )
            # multiply straight out of PSUM (a 2-semaphore wait costs one
            # cheap engine-sequencer event-semaphore, less than a copy)
            nc.vector.tensor_scalar_mul(
                of_all[:, b * ODIM : b * ODIM + DIM], psum_sums[:], recip[:]
            )

        # single contiguous output DMA: out_feat[m, b, :] <- of_all[m, b*ODIM:]
        nc.sync.dma_start(
            out=out_feat.rearrange("m b d -> m (b d)"), in_=of_all[:]
        )


    nc.finalize()
    return nc


def get_nc() -> bass.Bass:
    global _nc_cache
    if _nc_cache is None:
        _nc_cache = _build_nc()
    return _nc_cache


def make_in_maps(token_features: np.ndarray, line_idxes: np.ndarray):
    idx = np.ascontiguousarray(line_idxes.reshape(BSZ, SEQ), dtype=np.int32)
    feat = np.ascontiguousarray(token_features, dtype=np.float32)
    in_maps = []
    for c in range(N_CORES):
        sl = slice(c * B_PER_CORE, (c + 1) * B_PER_CORE)
        in_maps.append(
            {
                "feat": np.ascontiguousarray(feat[sl]),
                "idx": np.ascontiguousarray(idx[sl]),
            }
        )
    return in_maps


def assemble(results):
    # per-core result is [ML, B_PER_CORE, ODIM]; reorder to [bsz, ML, ODIM]
    out = np.concatenate(
        [r["out_feat"].transpose(1, 0, 2) for r in results], axis=0
    )
    line_features = np.ascontiguousarray(out[:, :, :DIM], dtype=np.float32)
    line_mask = out[:, :, DIM] != 0.0
    return line_features, line_mask


def kernel(**inputs) -> tuple[np.ndarray, np.ndarray]:
    token_features = np.asarray(inputs["token_features"])
    line_idxes = np.asarray(inputs["line_idxes"])
    nc = get_nc()
    in_maps = make_in_maps(token_features, line_idxes)
    res = run_bass_kernel_spmd(nc, in_maps, core_ids=list(range(N_CORES))).results
    return assemble(res)
